# revision 4
# baseline (speedup 1.0000x reference)
"""Trainium2 Bass kernel for nn_CumulativeIFFT.

Computes, for spectral (B=4, T=512, D=64, K=32, 2):
    s = spectral * sqrt(t+1)
    out[b,t,n,d] = (sum_k s_re[b,t,d,k]*cos(2pi n k/512)
                   - s_im[b,t,d,k]*sin(2pi n k/512)) / 512
Output: (4, 512, 512, 64) float32.

Formulation: per (b,t) pair, out[n,d] = sum_j WT[j,n] * Xt[j,d] where
j = 2k+ri flattens (k, re/im), WT folds cos/-sin and the 1/512, and
Xt = transpose(spectral[b,t]) * sqrt(t+1).

Sharding: 8 cores; core c handles b = c//2, t in [ (c%2)*256, (c%2)*256+256 ).
No cross-core communication.
"""

import math
import sys

import numpy as np

for _p in ("/opt/trn_rl_repo", "/root/.axon_site/_ro/trn_rl_repo"):
    if _p not in sys.path:
        sys.path.append(_p)

B, T, D, K = 4, 512, 64, 32
J = 2 * K          # flattened (k, re/im) contraction axis
N = 512            # output sequence length (seq_len)
NCORES = 8
TP = (B * T) // NCORES   # (b,t) pairs per core = 256
GP = 8                   # pairs per group
NG = TP // GP            # groups per core = 32
NB = N // 128            # 128-row output blocks = 4

_CACHE = {}


def _build_program():
    import concourse.bass as bass  # noqa: F401
    import concourse.tile as tile
    from concourse import bacc, mybir

    f32 = mybir.dt.float32
    nc = bacc.Bacc("TRN2")

    x = nc.dram_tensor("x", [TP, D, J], f32, kind="ExternalInput")
    wt = nc.dram_tensor("wt", [J, N], f32, kind="ExternalInput")
    scl = nc.dram_tensor("scl", [D, TP], f32, kind="ExternalInput")
    ident = nc.dram_tensor("ident", [D, D], f32, kind="ExternalInput")
    out = nc.dram_tensor("out", [TP, N, D], f32, kind="ExternalOutput")

    with tile.TileContext(nc) as tc:
        with (
            tc.tile_pool(name="const", bufs=1) as constp,
            tc.tile_pool(name="xin", bufs=3) as xinp,
            tc.tile_pool(name="xt", bufs=3) as xtp,
            tc.tile_pool(name="osb", bufs=3) as osbp,
            tc.tile_pool(name="pst", bufs=4, space="PSUM") as pstp,
            tc.tile_pool(name="pso", bufs=4, space="PSUM") as psop,
        ):
            wt_sb = constp.tile([J, N], f32)
            nc.sync.dma_start(wt_sb[:], wt[:])
            scl_sb = constp.tile([D, TP], f32)
            nc.sync.dma_start(scl_sb[:], scl[:])
            id_sb = constp.tile([D, D], f32)
            nc.sync.dma_start(id_sb[:], ident[:])

            for g in range(NG):
                # Load 8 pairs: DRAM [t][d][j] -> SBUF (d parts, (p, j) cols)
                xn = xinp.tile([D, GP, J], f32)
                nc.sync.dma_start(
                    xn[:], x[g * GP:(g + 1) * GP].transpose([1, 0, 2])
                )

                # Transpose each pair on PE, scale by sqrt(t+1) on copy-out
                xt = xtp.tile([J, GP, D], f32)
                for p in range(GP):
                    ps_t = pstp.tile([J, D], f32, tag="pst")
                    nc.tensor.transpose(ps_t[:], xn[:, p, :], id_sb[:])
                    ti = g * GP + p
                    nc.scalar.mul(xt[:, p, :], ps_t[:], scl_sb[:, ti:ti + 1])

                # Main GEMM: out_block[n, (p,d)] = WT_block.T @ Xt
                osb = osbp.tile([128, NB, GP, D], f32)
                for nb in range(NB):
                    ps_o = psop.tile([128, GP, D], f32, tag="pso")
                    nc.tensor.matmul(
                        ps_o[:],
                        wt_sb[:, nb * 128:(nb + 1) * 128],
                        xt[:],
                        start=True,
                        stop=True,
                    )
                    if nb < 3:
                        nc.vector.tensor_copy(osb[:, nb, :, :], ps_o[:])
                    else:
                        nc.scalar.copy(osb[:, nb, :, :], ps_o[:])

                # Store: per pair, (n parts, (nb, d)) -> DRAM [n*64+d] runs
                for p in range(GP):
                    nc.sync.dma_start(
                        out[g * GP + p].rearrange("(nb n) d -> n nb d", n=128),
                        osb[:, :, p, :],
                    )
    nc.compile()
    return nc


def _constants():
    n = np.arange(N, dtype=np.float32)
    k = np.arange(K, dtype=np.float32)
    ang = np.float32(2.0 * math.pi / N) * np.outer(n, k)  # (N, K) f32
    wt = np.empty((J, N), dtype=np.float32)
    wt[0::2, :] = (np.cos(ang) / N).T.astype(np.float32)
    wt[1::2, :] = (-np.sin(ang) / N).T.astype(np.float32)
    ident = np.eye(D, dtype=np.float32)
    return wt, ident


def _run(spectral: np.ndarray, trace: bool = False, **kw):
    from concourse import bass_utils

    spectral = np.ascontiguousarray(spectral, dtype=np.float32)
    assert spectral.shape == (B, T, D, K, 2)

    if "nc" not in _CACHE:
        _CACHE["nc"] = _build_program()
        _CACHE["consts"] = _constants()
    nc = _CACHE["nc"]
    wt, ident = _CACHE["consts"]

    thalf = T // 2
    in_maps = []
    for c in range(NCORES):
        b, t0 = c // 2, (c % 2) * thalf
        xc = np.ascontiguousarray(
            spectral[b, t0:t0 + thalf].reshape(TP, D, J)
        )
        sc = np.sqrt(np.arange(t0 + 1, t0 + TP + 1, dtype=np.float32))
        sclc = np.ascontiguousarray(
            np.broadcast_to(sc[None, :], (D, TP)).astype(np.float32)
        )
        in_maps.append({"x": xc, "wt": wt, "scl": sclc, "ident": ident})

    res = bass_utils.run_bass_kernel_spmd(
        nc, in_maps, core_ids=list(range(NCORES)), trace=trace, **kw
    )

    out = np.empty((B, T, N, D), dtype=np.float32)
    for c in range(NCORES):
        b, t0 = c // 2, (c % 2) * thalf
        out[b, t0:t0 + thalf] = res.results[c]["out"]
    return out, res


def kernel(spectral: np.ndarray) -> np.ndarray:
    return _run(spectral, trace=False)[0]


# revision 5
# speedup vs baseline: 1.2224x; 1.2224x over previous
"""Trainium2 Bass kernel for nn_CumulativeIFFT.

Computes, for spectral (B=4, T=512, D=64, K=32, 2):
    s = spectral * sqrt(t+1)
    out[b,t,n,d] = (sum_k s_re[b,t,d,k]*cos(2pi n k/512)
                   - s_im[b,t,d,k]*sin(2pi n k/512)) / 512
Output: (4, 512, 512, 64) float32.

Formulation: per (b,t) pair, out[n,d] = sum_j WT[j,n] * Xt[j,d] where
j = 2k+ri flattens (k, re/im), WT folds cos/-sin and the 1/512, and
Xt = transpose(spectral[b,t]) * sqrt(t+1).

Sharding: 8 cores; core c handles b = c//2, t in [ (c%2)*256, (c%2)*256+256 ).
No cross-core communication.
"""

import math
import sys

import numpy as np

for _p in ("/opt/trn_rl_repo", "/root/.axon_site/_ro/trn_rl_repo"):
    if _p not in sys.path:
        sys.path.append(_p)

B, T, D, K = 4, 512, 64, 32
J = 2 * K          # flattened (k, re/im) contraction axis
N = 512            # output sequence length (seq_len)
NCORES = 8
TP = (B * T) // NCORES   # (b,t) pairs per core = 256
GP = 8                   # pairs per group
NG = TP // GP            # groups per core = 32
NB = N // 128            # 128-row output blocks = 4

_CACHE = {}


def _build_program():
    import concourse.bass as bass  # noqa: F401
    import concourse.tile as tile
    from concourse import bacc, mybir

    f32 = mybir.dt.float32
    nc = bacc.Bacc("TRN2")

    x = nc.dram_tensor("x", [TP, D, J], f32, kind="ExternalInput")
    wt = nc.dram_tensor("wt", [J, N], f32, kind="ExternalInput")
    scl = nc.dram_tensor("scl", [D, TP], f32, kind="ExternalInput")
    ident = nc.dram_tensor("ident", [D, D], f32, kind="ExternalInput")
    out = nc.dram_tensor("out", [TP, N, D], f32, kind="ExternalOutput")

    with tile.TileContext(nc) as tc:
        with (
            tc.tile_pool(name="const", bufs=1) as constp,
            tc.tile_pool(name="xin", bufs=3) as xinp,
            tc.tile_pool(name="xt", bufs=3) as xtp,
            tc.tile_pool(name="osb", bufs=3) as osbp,
            tc.tile_pool(name="pst", bufs=4, space="PSUM") as pstp,
            tc.tile_pool(name="pso", bufs=4, space="PSUM") as psop,
        ):
            wt_sb = constp.tile([J, N], f32)
            nc.sync.dma_start(wt_sb[:], wt[:])
            scl_sb = constp.tile([D, TP], f32)
            nc.sync.dma_start(scl_sb[:], scl[:])
            id_sb = constp.tile([D, D], f32)
            nc.sync.dma_start(id_sb[:], ident[:])

            for g in range(NG):
                # Load 8 pairs: DRAM [t][d][j] -> SBUF (d parts, (p, j) cols)
                xn = xinp.tile([D, GP, J], f32)
                nc.sync.dma_start(
                    xn[:], x[g * GP:(g + 1) * GP].transpose([1, 0, 2])
                )

                # Transpose each pair on PE, scale by sqrt(t+1) on copy-out
                xt = xtp.tile([J, GP, D], f32)
                for p in range(GP):
                    ps_t = pstp.tile([J, D], f32, tag="pst")
                    nc.tensor.transpose(ps_t[:], xn[:, p, :], id_sb[:])
                    ti = g * GP + p
                    nc.scalar.mul(xt[:, p, :], ps_t[:], scl_sb[:, ti:ti + 1])

                # Main GEMM, n interleaved mod 4: matmul r computes rows
                # n = 4q + r into psum partition q, so each SBUF partition
                # ends up holding 4 consecutive n rows = 1024B DRAM runs.
                osb = osbp.tile([128, GP, NB, D], f32)
                for r in range(NB):
                    ps_o = psop.tile([128, GP, D], f32, tag="pso")
                    nc.tensor.matmul(
                        ps_o[:],
                        wt_sb[:, r::NB],
                        xt[:],
                        start=True,
                        stop=True,
                    )
                    if r < 2:
                        nc.vector.tensor_copy(osb[:, :, r, :], ps_o[:])
                    else:
                        nc.scalar.copy(osb[:, :, r, :], ps_o[:])

                # Store whole group: partition q covers n=4q..4q+3
                nc.sync.dma_start(
                    out[g * GP:(g + 1) * GP].rearrange(
                        "p (q r) d -> q p r d", r=NB
                    ),
                    osb[:],
                )
    nc.compile()
    return nc


def _constants():
    n = np.arange(N, dtype=np.float32)
    k = np.arange(K, dtype=np.float32)
    ang = np.float32(2.0 * math.pi / N) * np.outer(n, k)  # (N, K) f32
    wt = np.empty((J, N), dtype=np.float32)
    wt[0::2, :] = (np.cos(ang) / N).T.astype(np.float32)
    wt[1::2, :] = (-np.sin(ang) / N).T.astype(np.float32)
    ident = np.eye(D, dtype=np.float32)
    return wt, ident


def _run(spectral: np.ndarray, trace: bool = False, **kw):
    from concourse import bass_utils

    spectral = np.ascontiguousarray(spectral, dtype=np.float32)
    assert spectral.shape == (B, T, D, K, 2)

    if "nc" not in _CACHE:
        _CACHE["nc"] = _build_program()
        _CACHE["consts"] = _constants()
    nc = _CACHE["nc"]
    wt, ident = _CACHE["consts"]

    thalf = T // 2
    in_maps = []
    for c in range(NCORES):
        b, t0 = c // 2, (c % 2) * thalf
        xc = np.ascontiguousarray(
            spectral[b, t0:t0 + thalf].reshape(TP, D, J)
        )
        sc = np.sqrt(np.arange(t0 + 1, t0 + TP + 1, dtype=np.float32))
        sclc = np.ascontiguousarray(
            np.broadcast_to(sc[None, :], (D, TP)).astype(np.float32)
        )
        in_maps.append({"x": xc, "wt": wt, "scl": sclc, "ident": ident})

    res = bass_utils.run_bass_kernel_spmd(
        nc, in_maps, core_ids=list(range(NCORES)), trace=trace, **kw
    )

    out = np.empty((B, T, N, D), dtype=np.float32)
    for c in range(NCORES):
        b, t0 = c // 2, (c % 2) * thalf
        out[b, t0:t0 + thalf] = res.results[c]["out"]
    return out, res


def kernel(spectral: np.ndarray) -> np.ndarray:
    return _run(spectral, trace=False)[0]


# revision 10
# speedup vs baseline: 1.8730x; 1.5323x over previous
"""Trainium2 Bass kernel for nn_CumulativeIFFT.

Computes, for spectral (B=4, T=512, D=64, K=32, 2):
    s = spectral * sqrt(t+1)
    out[b,t,n,d] = (sum_k s_re[b,t,d,k]*cos(2pi n k/512)
                   - s_im[b,t,d,k]*sin(2pi n k/512)) / 512
Output: (4, 512, 512, 64) float32.

Formulation: per (b,t) pair, out[n,d] = sum_j WT[j,n] * Xt[j,d] where
j = 2k+ri flattens (k, re/im), WT folds cos/-sin and the 1/512, and
Xt = transpose(spectral[b,t]) * sqrt(t+1).

Sharding: 8 cores; core c handles b = c//2, t in [ (c%2)*256, (c%2)*256+256 ).
No cross-core communication.
"""

import math
import sys

import numpy as np

for _p in ("/opt/trn_rl_repo", "/root/.axon_site/_ro/trn_rl_repo"):
    if _p not in sys.path:
        sys.path.append(_p)

B, T, D, K = 4, 512, 64, 32
J = 2 * K          # flattened (k, re/im) contraction axis
N = 512            # output sequence length (seq_len)
NCORES = 8
TP = (B * T) // NCORES   # (b,t) pairs per core = 256
GP = 8                   # pairs per group
NG = TP // GP            # groups per core = 32
NB = N // 128            # 128-row output blocks = 4

_CACHE = {}


def _build_program():
    import concourse.bass as bass  # noqa: F401
    import concourse.tile as tile
    from concourse import bacc, mybir

    f32 = mybir.dt.float32
    f32r = mybir.dt.float32r
    nc = bacc.Bacc("TRN2")

    x = nc.dram_tensor("x", [TP, D, J], f32, kind="ExternalInput")
    wt = nc.dram_tensor("wt", [J, N], f32, kind="ExternalInput")
    scl = nc.dram_tensor("scl", [D, TP], f32, kind="ExternalInput")
    ident = nc.dram_tensor("ident", [D, D], f32, kind="ExternalInput")
    out = nc.dram_tensor("out", [TP, N, D], f32, kind="ExternalOutput")

    with tile.TileContext(nc) as tc:
        with (
            tc.tile_pool(name="const", bufs=1) as constp,
            tc.tile_pool(name="xin", bufs=3) as xinp,
            tc.tile_pool(name="xt", bufs=3) as xtp,
            tc.tile_pool(name="osb", bufs=3) as osbp,
            tc.tile_pool(name="pst", bufs=4, space="PSUM") as pstp,
            tc.tile_pool(name="pso", bufs=4, space="PSUM") as psop,
        ):
            wt_sb = constp.tile([J, N], f32)
            nc.sync.dma_start(wt_sb[:], wt[:])
            wt_r = constp.tile([J, N], f32r)
            nc.vector.tensor_copy(wt_r[:], wt_sb[:])
            scl_sb = constp.tile([D, TP], f32)
            nc.sync.dma_start(scl_sb[:], scl[:])
            id_sb = constp.tile([D, D], f32)
            nc.sync.dma_start(id_sb[:], ident[:])

            for g in range(NG):
                # Load 8 pairs: DRAM [t][d][j] -> SBUF (d parts, (p, j) cols)
                xn = xinp.tile([D, GP, J], f32)
                nc.sync.dma_start(
                    xn[:], x[g * GP:(g + 1) * GP].transpose([1, 0, 2])
                )

                # Transpose each pair on PE, scale by sqrt(t+1) on copy-out
                xt = xtp.tile([J, GP, D], f32r)
                for p in range(GP):
                    ps_t = pstp.tile([J, D], f32, tag="pst")
                    nc.tensor.transpose(ps_t[:], xn[:, p, :], id_sb[:])
                    ti = g * GP + p
                    nc.scalar.mul(xt[:, p, :], ps_t[:], scl_sb[:, ti:ti + 1])

                # Main GEMM, n interleaved mod 4: matmul r computes rows
                # n = 4q + r into psum partition q, so each SBUF partition
                # ends up holding 4 consecutive n rows = 1024B DRAM runs.
                osb = osbp.tile([128, GP, NB, D], f32)
                for r in range(NB):
                    ps_o = psop.tile([128, GP, D], f32, tag="pso")
                    nc.tensor.matmul(
                        ps_o[:],
                        wt_r[:, r::NB],
                        xt[:],
                        start=True,
                        stop=True,
                    )
                    if r < 2:
                        nc.vector.tensor_copy(osb[:, :, r, :], ps_o[:])
                    else:
                        nc.scalar.copy(osb[:, :, r, :], ps_o[:])

                # Store whole group: partition q covers n=4q..4q+3
                nc.sync.dma_start(
                    out[g * GP:(g + 1) * GP].rearrange(
                        "p (q r) d -> q p r d", r=NB
                    ),
                    osb[:],
                )
    nc.compile()
    return nc


def _constants():
    n = np.arange(N, dtype=np.float32)
    k = np.arange(K, dtype=np.float32)
    ang = np.float32(2.0 * math.pi / N) * np.outer(n, k)  # (N, K) f32
    wt = np.empty((J, N), dtype=np.float32)
    wt[0::2, :] = (np.cos(ang) / N).T.astype(np.float32)
    wt[1::2, :] = (-np.sin(ang) / N).T.astype(np.float32)
    ident = np.eye(D, dtype=np.float32)
    return wt, ident


def _run(spectral: np.ndarray, trace: bool = False, **kw):
    from concourse import bass_utils

    spectral = np.ascontiguousarray(spectral, dtype=np.float32)
    assert spectral.shape == (B, T, D, K, 2)

    if "nc" not in _CACHE:
        _CACHE["nc"] = _build_program()
        _CACHE["consts"] = _constants()
    nc = _CACHE["nc"]
    wt, ident = _CACHE["consts"]

    thalf = T // 2
    in_maps = []
    for c in range(NCORES):
        b, t0 = c // 2, (c % 2) * thalf
        xc = np.ascontiguousarray(
            spectral[b, t0:t0 + thalf].reshape(TP, D, J)
        )
        sc = np.sqrt(np.arange(t0 + 1, t0 + TP + 1, dtype=np.float32))
        sclc = np.ascontiguousarray(
            np.broadcast_to(sc[None, :], (D, TP)).astype(np.float32)
        )
        in_maps.append({"x": xc, "wt": wt, "scl": sclc, "ident": ident})

    res = bass_utils.run_bass_kernel_spmd(
        nc, in_maps, core_ids=list(range(NCORES)), trace=trace, **kw
    )

    out = np.empty((B, T, N, D), dtype=np.float32)
    for c in range(NCORES):
        b, t0 = c // 2, (c % 2) * thalf
        out[b, t0:t0 + thalf] = res.results[c]["out"]
    return out, res


def kernel(spectral: np.ndarray) -> np.ndarray:
    return _run(spectral, trace=False)[0]


# revision 13
# speedup vs baseline: 2.4160x; 1.2899x over previous
"""Trainium2 Bass kernel for nn_CumulativeIFFT.

Computes, for spectral (B=4, T=512, D=64, K=32, 2):
    s = spectral * sqrt(t+1)
    out[b,t,n,d] = (sum_k s_re[b,t,d,k]*cos(2pi n k/512)
                   - s_im[b,t,d,k]*sin(2pi n k/512)) / 512
Output: (4, 512, 512, 64) float32.

Formulation: per (b,t) pair, out[n,d] = sum_j WT[j,n] * Xt[j,d] where
j = 2k+ri flattens (k, re/im), WT folds cos/-sin and the 1/512, and
Xt = transpose(spectral[b,t]) * sqrt(t+1).

Sharding: 8 cores; core c handles b = c//2, t in [ (c%2)*256, (c%2)*256+256 ).
No cross-core communication.
"""

import math
import sys

import numpy as np

for _p in ("/opt/trn_rl_repo", "/root/.axon_site/_ro/trn_rl_repo"):
    if _p not in sys.path:
        sys.path.append(_p)

B, T, D, K = 4, 512, 64, 32
J = 2 * K          # flattened (k, re/im) contraction axis
N = 512            # output sequence length (seq_len)
NCORES = 8
TP = (B * T) // NCORES   # (b,t) pairs per core = 256
GP = 8                   # pairs per group
NG = TP // GP            # groups per core = 32
NB = N // 128            # 128-row output blocks = 4

_CACHE = {}


def _build_program():
    import concourse.bass as bass  # noqa: F401
    import concourse.tile as tile
    from concourse import bacc, mybir

    f32 = mybir.dt.float32
    f32r = mybir.dt.float32r
    nc = bacc.Bacc("TRN2")

    x = nc.dram_tensor("x", [TP, D, J], f32, kind="ExternalInput")
    wt = nc.dram_tensor("wt", [J, N], f32, kind="ExternalInput")
    scl = nc.dram_tensor("scl", [D, TP], f32, kind="ExternalInput")
    ident = nc.dram_tensor("ident", [2 * D, 2 * D], f32, kind="ExternalInput")
    out = nc.dram_tensor("out", [TP, N, D], f32, kind="ExternalOutput")

    with tile.TileContext(nc) as tc:
        with (
            tc.tile_pool(name="const", bufs=1) as constp,
            tc.tile_pool(name="xin", bufs=3) as xinp,
            tc.tile_pool(name="xt", bufs=3) as xtp,
            tc.tile_pool(name="osb", bufs=3) as osbp,
            tc.tile_pool(name="pst", bufs=4, space="PSUM") as pstp,
            tc.tile_pool(name="pso", bufs=4, space="PSUM") as psop,
        ):
            wt_sb = constp.tile([J, N], f32)
            nc.sync.dma_start(wt_sb[:], wt[:])
            wt_r = constp.tile([J, N], f32r)
            nc.vector.tensor_copy(wt_r[:], wt_sb[:])
            scl_sb = constp.tile([D, TP], f32)
            nc.sync.dma_start(scl_sb[:], scl[:])
            id_sb = constp.tile([2 * D, 2 * D], f32)
            nc.sync.dma_start(id_sb[:], ident[:])

            for g in range(NG):
                # Load 8 pairs: DRAM [t][d][j] -> SBUF ((v,d) parts, (u, j))
                # with pair p = 2u+v, so one PE transpose handles 2 pairs.
                xn = xinp.tile([2 * D, GP // 2, J], f32)
                nc.sync.dma_start(
                    xn[:],
                    x[g * GP:(g + 1) * GP].rearrange(
                        "(u v) d j -> (v d) u j", v=2
                    ),
                )

                # PE transpose 2 pairs at a time -> psum (j, (u, v, d))
                ps_t = pstp.tile([J, GP, D], f32, tag="pst")
                for u in range(GP // 2):
                    nc.tensor.transpose(
                        ps_t[:, 2 * u:2 * u + 2, :], xn[:, u, :], id_sb[:]
                    )

                # One broadcast multiply: xt = ps_t * sqrt(t+1) (per pair)
                xt = xtp.tile([J, GP, D], f32r)
                scl_b = (
                    scl_sb[:, g * GP:(g + 1) * GP]
                    .unsqueeze(-1)
                    .broadcast_to([D, GP, D])
                )
                nc.vector.tensor_mul(xt[:], ps_t[:], scl_b)

                # Main GEMM, n interleaved mod 4: matmul r computes rows
                # n = 4q + r into psum partition q, so each SBUF partition
                # ends up holding 4 consecutive n rows = 1024B DRAM runs.
                osb = osbp.tile([128, GP, NB, D], f32)
                for r in range(NB):
                    ps_o = psop.tile([128, GP, D], f32, tag="pso")
                    nc.tensor.matmul(
                        ps_o[:],
                        wt_r[:, r::NB],
                        xt[:],
                        start=True,
                        stop=True,
                    )
                    if r < 2:
                        nc.vector.tensor_copy(osb[:, :, r, :], ps_o[:])
                    else:
                        nc.scalar.copy(osb[:, :, r, :], ps_o[:])

                # Store whole group: partition q covers n=4q..4q+3
                nc.sync.dma_start(
                    out[g * GP:(g + 1) * GP].rearrange(
                        "p (q r) d -> q p r d", r=NB
                    ),
                    osb[:],
                )
    nc.compile()
    return nc


def _constants():
    n = np.arange(N, dtype=np.float32)
    k = np.arange(K, dtype=np.float32)
    ang = np.float32(2.0 * math.pi / N) * np.outer(n, k)  # (N, K) f32
    wt = np.empty((J, N), dtype=np.float32)
    wt[0::2, :] = (np.cos(ang) / N).T.astype(np.float32)
    wt[1::2, :] = (-np.sin(ang) / N).T.astype(np.float32)
    ident = np.eye(2 * D, dtype=np.float32)
    return wt, ident


def _run(spectral: np.ndarray, trace: bool = False, **kw):
    from concourse import bass_utils

    spectral = np.ascontiguousarray(spectral, dtype=np.float32)
    assert spectral.shape == (B, T, D, K, 2)

    if "nc" not in _CACHE:
        _CACHE["nc"] = _build_program()
        _CACHE["consts"] = _constants()
    nc = _CACHE["nc"]
    wt, ident = _CACHE["consts"]

    thalf = T // 2
    in_maps = []
    for c in range(NCORES):
        b, t0 = c // 2, (c % 2) * thalf
        xc = np.ascontiguousarray(
            spectral[b, t0:t0 + thalf].reshape(TP, D, J)
        )
        sc = np.sqrt(np.arange(t0 + 1, t0 + TP + 1, dtype=np.float32))
        sclc = np.ascontiguousarray(
            np.broadcast_to(sc[None, :], (D, TP)).astype(np.float32)
        )
        in_maps.append({"x": xc, "wt": wt, "scl": sclc, "ident": ident})

    res = bass_utils.run_bass_kernel_spmd(
        nc, in_maps, core_ids=list(range(NCORES)), trace=trace, **kw
    )

    out = np.empty((B, T, N, D), dtype=np.float32)
    for c in range(NCORES):
        b, t0 = c // 2, (c % 2) * thalf
        out[b, t0:t0 + thalf] = res.results[c]["out"]
    return out, res


def kernel(spectral: np.ndarray) -> np.ndarray:
    return _run(spectral, trace=False)[0]


# revision 14
# speedup vs baseline: 2.4317x; 1.0065x over previous
"""Trainium2 Bass kernel for nn_CumulativeIFFT.

Computes, for spectral (B=4, T=512, D=64, K=32, 2):
    s = spectral * sqrt(t+1)
    out[b,t,n,d] = (sum_k s_re[b,t,d,k]*cos(2pi n k/512)
                   - s_im[b,t,d,k]*sin(2pi n k/512)) / 512
Output: (4, 512, 512, 64) float32.

Formulation: per (b,t) pair, out[n,d] = sum_j WT[j,n] * Xt[j,d] where
j = 2k+ri flattens (k, re/im), WT folds cos/-sin and the 1/512, and
Xt = transpose(spectral[b,t]) * sqrt(t+1).

Sharding: 8 cores; core c handles b = c//2, t in [ (c%2)*256, (c%2)*256+256 ).
No cross-core communication.
"""

import math
import sys

import numpy as np

for _p in ("/opt/trn_rl_repo", "/root/.axon_site/_ro/trn_rl_repo"):
    if _p not in sys.path:
        sys.path.append(_p)

B, T, D, K = 4, 512, 64, 32
J = 2 * K          # flattened (k, re/im) contraction axis
N = 512            # output sequence length (seq_len)
NCORES = 8
TP = (B * T) // NCORES   # (b,t) pairs per core = 256
GP = 8                   # pairs per group
NG = TP // GP            # groups per core = 32
NB = N // 128            # 128-row output blocks = 4

_CACHE = {}


def _build_program():
    import concourse.bass as bass  # noqa: F401
    import concourse.tile as tile
    from concourse import bacc, mybir

    f32 = mybir.dt.float32
    f32r = mybir.dt.float32r
    nc = bacc.Bacc("TRN2")

    x = nc.dram_tensor("x", [TP, D, J], f32, kind="ExternalInput")
    wt = nc.dram_tensor("wt", [J, N], f32, kind="ExternalInput")
    scl = nc.dram_tensor("scl", [D, TP], f32, kind="ExternalInput")
    ident = nc.dram_tensor("ident", [2 * D, 2 * D], f32, kind="ExternalInput")
    out = nc.dram_tensor("out", [TP, N, D], f32, kind="ExternalOutput")

    with tile.TileContext(nc) as tc:
        with (
            tc.tile_pool(name="const", bufs=1) as constp,
            tc.tile_pool(name="xin", bufs=4) as xinp,
            tc.tile_pool(name="xt", bufs=4) as xtp,
            tc.tile_pool(name="osb", bufs=5) as osbp,
            tc.tile_pool(name="pst", bufs=2, space="PSUM") as pstp,
            tc.tile_pool(name="pso", bufs=6, space="PSUM") as psop,
        ):
            wt_sb = constp.tile([J, N], f32)
            nc.sync.dma_start(wt_sb[:], wt[:])
            wt_r = constp.tile([J, N], f32r)
            nc.vector.tensor_copy(wt_r[:], wt_sb[:])
            scl_sb = constp.tile([D, TP], f32)
            nc.sync.dma_start(scl_sb[:], scl[:])
            id_sb = constp.tile([2 * D, 2 * D], f32)
            nc.sync.dma_start(id_sb[:], ident[:])

            for g in range(NG):
                # Load 8 pairs: DRAM [t][d][j] -> SBUF ((v,d) parts, (u, j))
                # with pair p = 2u+v, so one PE transpose handles 2 pairs.
                xn = xinp.tile([2 * D, GP // 2, J], f32)
                nc.sync.dma_start(
                    xn[:],
                    x[g * GP:(g + 1) * GP].rearrange(
                        "(u v) d j -> (v d) u j", v=2
                    ),
                )

                # PE transpose 2 pairs at a time -> psum (j, (u, v, d))
                ps_t = pstp.tile([J, GP, D], f32, tag="pst")
                for u in range(GP // 2):
                    nc.tensor.transpose(
                        ps_t[:, 2 * u:2 * u + 2, :], xn[:, u, :], id_sb[:]
                    )

                # One broadcast multiply: xt = ps_t * sqrt(t+1) (per pair)
                xt = xtp.tile([J, GP, D], f32r)
                scl_b = (
                    scl_sb[:, g * GP:(g + 1) * GP]
                    .unsqueeze(-1)
                    .broadcast_to([D, GP, D])
                )
                nc.vector.tensor_mul(xt[:], ps_t[:], scl_b)

                # Main GEMM, n interleaved mod 4: matmul r computes rows
                # n = 4q + r into psum partition q, so each SBUF partition
                # ends up holding 4 consecutive n rows = 1024B DRAM runs.
                osb = osbp.tile([128, GP, NB, D], f32)
                for r in range(NB):
                    ps_o = psop.tile([128, GP, D], f32, tag="pso")
                    nc.tensor.matmul(
                        ps_o[:],
                        wt_r[:, r::NB],
                        xt[:],
                        start=True,
                        stop=True,
                    )
                    if r < 2:
                        nc.vector.tensor_copy(osb[:, :, r, :], ps_o[:])
                    else:
                        nc.scalar.copy(osb[:, :, r, :], ps_o[:])

                # Store whole group: partition q covers n=4q..4q+3
                nc.sync.dma_start(
                    out[g * GP:(g + 1) * GP].rearrange(
                        "p (q r) d -> q p r d", r=NB
                    ),
                    osb[:],
                )
    nc.compile()
    return nc


def _constants():
    n = np.arange(N, dtype=np.float32)
    k = np.arange(K, dtype=np.float32)
    ang = np.float32(2.0 * math.pi / N) * np.outer(n, k)  # (N, K) f32
    wt = np.empty((J, N), dtype=np.float32)
    wt[0::2, :] = (np.cos(ang) / N).T.astype(np.float32)
    wt[1::2, :] = (-np.sin(ang) / N).T.astype(np.float32)
    ident = np.eye(2 * D, dtype=np.float32)
    return wt, ident


def _run(spectral: np.ndarray, trace: bool = False, **kw):
    from concourse import bass_utils

    spectral = np.ascontiguousarray(spectral, dtype=np.float32)
    assert spectral.shape == (B, T, D, K, 2)

    if "nc" not in _CACHE:
        _CACHE["nc"] = _build_program()
        _CACHE["consts"] = _constants()
    nc = _CACHE["nc"]
    wt, ident = _CACHE["consts"]

    thalf = T // 2
    in_maps = []
    for c in range(NCORES):
        b, t0 = c // 2, (c % 2) * thalf
        xc = np.ascontiguousarray(
            spectral[b, t0:t0 + thalf].reshape(TP, D, J)
        )
        sc = np.sqrt(np.arange(t0 + 1, t0 + TP + 1, dtype=np.float32))
        sclc = np.ascontiguousarray(
            np.broadcast_to(sc[None, :], (D, TP)).astype(np.float32)
        )
        in_maps.append({"x": xc, "wt": wt, "scl": sclc, "ident": ident})

    res = bass_utils.run_bass_kernel_spmd(
        nc, in_maps, core_ids=list(range(NCORES)), trace=trace, **kw
    )

    out = np.empty((B, T, N, D), dtype=np.float32)
    for c in range(NCORES):
        b, t0 = c // 2, (c % 2) * thalf
        out[b, t0:t0 + thalf] = res.results[c]["out"]
    return out, res


def kernel(spectral: np.ndarray) -> np.ndarray:
    return _run(spectral, trace=False)[0]


# revision 25
# speedup vs baseline: 2.7109x; 1.1148x over previous
"""Trainium2 Bass kernel for nn_CumulativeIFFT.

Computes, for spectral (B=4, T=512, D=64, K=32, 2):
    s = spectral * sqrt(t+1)
    out[b,t,n,d] = (sum_k s_re[b,t,d,k]*cos(2pi n k/512)
                   - s_im[b,t,d,k]*sin(2pi n k/512)) / 512
Output: (4, 512, 512, 64) float32.

Formulation: per (b,t) pair, out[n,d] = sum_j WT[j,n] * Xt[j,d] where
j = 2k+ri flattens (k, re/im), WT folds cos/-sin and the 1/512, and
Xt = transpose(spectral[b,t]) * sqrt(t+1).

Sharding: 8 cores; core c handles b = c//2, t in [ (c%2)*256, (c%2)*256+256 ).
No cross-core communication.
"""

import math
import sys

import numpy as np

for _p in ("/opt/trn_rl_repo", "/root/.axon_site/_ro/trn_rl_repo"):
    if _p not in sys.path:
        sys.path.append(_p)

B, T, D, K = 4, 512, 64, 32
J = 2 * K          # flattened (k, re/im) contraction axis
N = 512            # output sequence length (seq_len)
NCORES = 8
TP = (B * T) // NCORES   # (b,t) pairs per core = 256
GP = 8                   # pairs per group
NG = TP // GP            # groups per core = 32
NB = N // 128            # 128-row output blocks = 4

_CACHE = {}


def _build_program():
    import concourse.bass as bass  # noqa: F401
    import concourse.tile as tile
    from concourse import bacc, mybir

    f32 = mybir.dt.float32
    f32r = mybir.dt.float32r
    f16 = mybir.dt.float16
    bf16 = mybir.dt.bfloat16
    nc = bacc.Bacc("TRN2")

    x = nc.dram_tensor("x", [TP, D, J], f32, kind="ExternalInput")
    wt = nc.dram_tensor("wt", [J, N], f32, kind="ExternalInput")
    scl = nc.dram_tensor("scl", [2 * D, TP // 2], f32, kind="ExternalInput")
    ident = nc.dram_tensor("ident", [2 * D, 2 * D], f32, kind="ExternalInput")
    out = nc.dram_tensor("out", [TP, N, D], f32, kind="ExternalOutput")

    with tile.TileContext(nc) as tc:
        with (
            tc.tile_pool(name="const", bufs=1) as constp,
            tc.tile_pool(name="xin", bufs=4) as xinp,
            tc.tile_pool(name="xc", bufs=4) as xcp,
            tc.tile_pool(name="xt", bufs=4) as xtp,
            tc.tile_pool(name="osb", bufs=5) as osbp,
            tc.tile_pool(name="pst", bufs=2, space="PSUM") as pstp,
            tc.tile_pool(name="pso", bufs=6, space="PSUM") as psop,
        ):
            wt_sb = constp.tile([J, N], f32)
            nc.sync.dma_start(wt_sb[:], wt[:])
            wt_r = constp.tile([J, N], f16)
            nc.vector.tensor_copy(wt_r[:], wt_sb[:])
            scl_sb = constp.tile([2 * D, TP // 2], f32)
            nc.sync.dma_start(scl_sb[:], scl[:])
            id_sb = constp.tile([2 * D, 2 * D], f32)
            nc.sync.dma_start(id_sb[:], ident[:])
            id_h = constp.tile([2 * D, 2 * D], f16)
            nc.vector.tensor_copy(id_h[:], id_sb[:])

            for g in range(NG):
                # Load 8 pairs: DRAM [t][d][j] -> SBUF ((v,d) parts, (u, j))
                # with pair p = 2u+v, so one PE transpose handles 2 pairs.
                xn = xinp.tile([2 * D, GP // 2, J], f32)
                nc.sync.dma_start(
                    xn[:],
                    x[g * GP:(g + 1) * GP].rearrange(
                        "(u v) d j -> (v d) u j", v=2
                    ),
                )

                # Scale by sqrt(t+1) and round to fp16 in one DVE pass.
                # scl value depends on (v, u) = partition half x column block.
                xc = xcp.tile([2 * D, GP // 2, J], f16)
                scl_b = (
                    scl_sb[:, g * (GP // 2):(g + 1) * (GP // 2)]
                    .unsqueeze(-1)
                    .broadcast_to([2 * D, GP // 2, J])
                )
                nc.vector.tensor_mul(xc[:], xn[:], scl_b)

                # PE transpose 2 pairs at a time -> psum (j, (u, v, d)) fp16
                ps_t = pstp.tile([J, GP, D], f16, tag="pst")
                for u in range(GP // 2):
                    nc.tensor.transpose(
                        ps_t[:, 2 * u:2 * u + 2, :], xc[:, u, :], id_h[:]
                    )

                # PSUM -> SBUF so the main matmuls can read it.
                xt = xtp.tile([J, GP, D], f16)
                nc.scalar.copy(xt[:], ps_t[:])

                # Main GEMM, n interleaved mod 4: matmul r computes rows
                # n = 4q + r into psum partition q, so each SBUF partition
                # ends up holding 4 consecutive n rows = 1024B DRAM runs.
                osb = osbp.tile([128, GP, NB, D], f32)
                for r in range(NB):
                    ps_o = psop.tile([128, GP, D], f32, tag="pso")
                    nc.tensor.matmul(
                        ps_o[:],
                        wt_r[:, r::NB],
                        xt[:],
                        start=True,
                        stop=True,
                    )
                    if r < 2:
                        nc.vector.tensor_copy(osb[:, :, r, :], ps_o[:])
                    else:
                        nc.scalar.copy(osb[:, :, r, :], ps_o[:])

                # Store whole group: partition q covers n=4q..4q+3
                nc.sync.dma_start(
                    out[g * GP:(g + 1) * GP].rearrange(
                        "p (q r) d -> q p r d", r=NB
                    ),
                    osb[:],
                )
    nc.compile()
    return nc


def _constants():
    n = np.arange(N, dtype=np.float32)
    k = np.arange(K, dtype=np.float32)
    ang = np.float32(2.0 * math.pi / N) * np.outer(n, k)  # (N, K) f32
    wt = np.empty((J, N), dtype=np.float32)
    wt[0::2, :] = (np.cos(ang) / N).T.astype(np.float32)
    wt[1::2, :] = (-np.sin(ang) / N).T.astype(np.float32)
    ident = np.eye(2 * D, dtype=np.float32)
    return wt, ident


def _run(spectral: np.ndarray, trace: bool = False, **kw):
    from concourse import bass_utils

    spectral = np.ascontiguousarray(spectral, dtype=np.float32)
    assert spectral.shape == (B, T, D, K, 2)

    if "nc" not in _CACHE:
        _CACHE["nc"] = _build_program()
        _CACHE["consts"] = _constants()
    nc = _CACHE["nc"]
    wt, ident = _CACHE["consts"]

    thalf = T // 2
    in_maps = []
    for c in range(NCORES):
        b, t0 = c // 2, (c % 2) * thalf
        xc = np.ascontiguousarray(
            spectral[b, t0:t0 + thalf].reshape(TP, D, J)
        )
        # scl[(v,d), g*4+u] = sqrt(t0 + 8g + 2u + v + 1); pair p = 2u+v
        rows_v = (np.arange(2 * D) // D)[:, None]
        cols = np.arange(TP // 2)[None, :]
        tt = 8 * (cols // 4) + 2 * (cols % 4) + rows_v
        sclc = np.sqrt((t0 + tt + 1).astype(np.float32))
        in_maps.append({"x": xc, "wt": wt, "scl": sclc, "ident": ident})

    res = bass_utils.run_bass_kernel_spmd(
        nc, in_maps, core_ids=list(range(NCORES)), trace=trace, **kw
    )

    out = np.empty((B, T, N, D), dtype=np.float32)
    for c in range(NCORES):
        b, t0 = c // 2, (c % 2) * thalf
        out[b, t0:t0 + thalf] = res.results[c]["out"]
    return out, res


def kernel(spectral: np.ndarray) -> np.ndarray:
    return _run(spectral, trace=False)[0]


# revision 28
# speedup vs baseline: 2.7806x; 1.0257x over previous
"""Trainium2 Bass kernel for nn_CumulativeIFFT.

Computes, for spectral (B=4, T=512, D=64, K=32, 2):
    s = spectral * sqrt(t+1)
    out[b,t,n,d] = (sum_k s_re[b,t,d,k]*cos(2pi n k/512)
                   - s_im[b,t,d,k]*sin(2pi n k/512)) / 512
Output: (4, 512, 512, 64) float32.

Formulation: per (b,t) pair, out[n,d] = sum_j WT[j,n] * Xt[j,d] where
j = 2k+ri flattens (k, re/im), WT folds cos/-sin and the 1/512, and
Xt = transpose(spectral[b,t]) * sqrt(t+1).

Sharding: 8 cores; core c handles b = c//2, t in [ (c%2)*256, (c%2)*256+256 ).
No cross-core communication.
"""

import math
import sys

import numpy as np

for _p in ("/opt/trn_rl_repo", "/root/.axon_site/_ro/trn_rl_repo"):
    if _p not in sys.path:
        sys.path.append(_p)

B, T, D, K = 4, 512, 64, 32
J = 2 * K          # flattened (k, re/im) contraction axis
N = 512            # output sequence length (seq_len)
NCORES = 8
TP = (B * T) // NCORES   # (b,t) pairs per core = 256
GP = 8                   # pairs per group
NG = TP // GP            # groups per core = 32
NB = N // 128            # 128-row output blocks = 4

_CACHE = {}


def _build_program():
    import concourse.bass as bass  # noqa: F401
    import concourse.tile as tile
    from concourse import bacc, mybir

    f32 = mybir.dt.float32
    f32r = mybir.dt.float32r
    f16 = mybir.dt.float16
    bf16 = mybir.dt.bfloat16
    nc = bacc.Bacc("TRN2")

    x = nc.dram_tensor("x", [TP, D, J], f16, kind="ExternalInput")
    wt = nc.dram_tensor("wt", [J, N], f32, kind="ExternalInput")
    scl = nc.dram_tensor("scl", [2 * D, TP // 2], f32, kind="ExternalInput")
    ident = nc.dram_tensor("ident", [2 * D, 2 * D], f32, kind="ExternalInput")
    out = nc.dram_tensor("out", [TP, N, D], f32, kind="ExternalOutput")

    with tile.TileContext(nc) as tc:
        with (
            tc.tile_pool(name="const", bufs=1) as constp,
            tc.tile_pool(name="xin", bufs=4) as xinp,
            tc.tile_pool(name="xc", bufs=4) as xcp,
            tc.tile_pool(name="xt", bufs=4) as xtp,
            tc.tile_pool(name="osb", bufs=5) as osbp,
            tc.tile_pool(name="pst", bufs=2, space="PSUM") as pstp,
            tc.tile_pool(name="pso", bufs=6, space="PSUM") as psop,
        ):
            wt_sb = constp.tile([J, N], f32)
            nc.sync.dma_start(wt_sb[:], wt[:])
            wt_r = constp.tile([J, N], f16)
            nc.vector.tensor_copy(wt_r[:], wt_sb[:])
            scl_sb = constp.tile([2 * D, TP // 2], f32)
            nc.sync.dma_start(scl_sb[:], scl[:])
            id_sb = constp.tile([2 * D, 2 * D], f32)
            nc.sync.dma_start(id_sb[:], ident[:])
            id_h = constp.tile([2 * D, 2 * D], f16)
            nc.vector.tensor_copy(id_h[:], id_sb[:])

            for g in range(NG):
                # Load 8 pairs: DRAM [t][d][j] -> SBUF ((v,d) parts, (u, j))
                # with pair p = 2u+v, so one PE transpose handles 2 pairs.
                xn = xinp.tile([2 * D, GP // 2, J], f16)
                nc.sync.dma_start(
                    xn[:],
                    x[g * GP:(g + 1) * GP].rearrange(
                        "(u v) d j -> (v d) u j", v=2
                    ),
                )

                # Scale by sqrt(t+1) and round to fp16 in one DVE pass.
                # scl value depends on (v, u) = partition half x column block.
                xc = xcp.tile([2 * D, GP // 2, J], f16)
                scl_b = (
                    scl_sb[:, g * (GP // 2):(g + 1) * (GP // 2)]
                    .unsqueeze(-1)
                    .broadcast_to([2 * D, GP // 2, J])
                )
                nc.vector.tensor_mul(xc[:], xn[:], scl_b)

                # PE transpose 2 pairs at a time -> psum (j, (u, v, d)) fp16
                ps_t = pstp.tile([J, GP, D], f16, tag="pst")
                for u in range(GP // 2):
                    nc.tensor.transpose(
                        ps_t[:, 2 * u:2 * u + 2, :], xc[:, u, :], id_h[:]
                    )

                # PSUM -> SBUF so the main matmuls can read it.
                xt = xtp.tile([J, GP, D], f16)
                nc.scalar.copy(xt[:], ps_t[:])

                # Main GEMM, n interleaved mod 4: matmul r computes rows
                # n = 4q + r into psum partition q, so each SBUF partition
                # ends up holding 4 consecutive n rows = 1024B DRAM runs.
                osb = osbp.tile([128, GP, NB, D], f32)
                for r in range(NB):
                    ps_o = psop.tile([128, GP, D], f32, tag="pso")
                    nc.tensor.matmul(
                        ps_o[:],
                        wt_r[:, r::NB],
                        xt[:],
                        start=True,
                        stop=True,
                    )
                    if r < 2:
                        nc.vector.tensor_copy(osb[:, :, r, :], ps_o[:])
                    else:
                        nc.scalar.copy(osb[:, :, r, :], ps_o[:])

                # Store whole group: partition q covers n=4q..4q+3
                nc.sync.dma_start(
                    out[g * GP:(g + 1) * GP].rearrange(
                        "p (q r) d -> q p r d", r=NB
                    ),
                    osb[:],
                )
    nc.compile()
    return nc


def _constants():
    n = np.arange(N, dtype=np.float32)
    k = np.arange(K, dtype=np.float32)
    ang = np.float32(2.0 * math.pi / N) * np.outer(n, k)  # (N, K) f32
    wt = np.empty((J, N), dtype=np.float32)
    wt[0::2, :] = (np.cos(ang) / N).T.astype(np.float32)
    wt[1::2, :] = (-np.sin(ang) / N).T.astype(np.float32)
    ident = np.eye(2 * D, dtype=np.float32)
    return wt, ident


def _run(spectral: np.ndarray, trace: bool = False, **kw):
    from concourse import bass_utils

    spectral = np.ascontiguousarray(spectral, dtype=np.float32)
    assert spectral.shape == (B, T, D, K, 2)

    if "nc" not in _CACHE:
        _CACHE["nc"] = _build_program()
        _CACHE["consts"] = _constants()
    nc = _CACHE["nc"]
    wt, ident = _CACHE["consts"]

    thalf = T // 2
    in_maps = []
    for c in range(NCORES):
        b, t0 = c // 2, (c % 2) * thalf
        xc = np.ascontiguousarray(
            spectral[b, t0:t0 + thalf].reshape(TP, D, J).astype(np.float16)
        )
        # scl[(v,d), g*4+u] = sqrt(t0 + 8g + 2u + v + 1); pair p = 2u+v
        rows_v = (np.arange(2 * D) // D)[:, None]
        cols = np.arange(TP // 2)[None, :]
        tt = 8 * (cols // 4) + 2 * (cols % 4) + rows_v
        sclc = np.sqrt((t0 + tt + 1).astype(np.float32))
        in_maps.append({"x": xc, "wt": wt, "scl": sclc, "ident": ident})

    res = bass_utils.run_bass_kernel_spmd(
        nc, in_maps, core_ids=list(range(NCORES)), trace=trace, **kw
    )

    out = np.empty((B, T, N, D), dtype=np.float32)
    for c in range(NCORES):
        b, t0 = c // 2, (c % 2) * thalf
        out[b, t0:t0 + thalf] = res.results[c]["out"]
    return out, res


def kernel(spectral: np.ndarray) -> np.ndarray:
    return _run(spectral, trace=False)[0]
